# revision 1
# baseline (speedup 1.0000x reference)
"""Trainium2 Bass kernel for multi-head attention (B=4, T=2048, HID=1024, H=16, D=64).

Sharding (8 NeuronCores): core c owns batch b = c//2 and query rows
g = c%2 (1024 of 2048). Each core projects q/k/v for its own 1024 rows;
the k/v projections are exchanged inside the (2b, 2b+1) core pair with a
2-core AllGather so every core attends over the full 2048 keys of its batch.
The final output projection is row-parallel, so the per-core outputs tile the
full [4, 2048, 1024] result with no reduction.

Numerics: every matmul runs in float32r (full-rate reduced-precision fp32 PE
mode, ~1.5e-4 per-matmul relative error). Attention probabilities are computed
by the Scalar engine directly out of PSUM with exp(S/8 + bias), the -1e9
pad-mask bias folded into the per-partition bias operand. The softmax
denominator comes from a ones-column appended to the V operand of the
probability @ V matmul, and normalization happens on the small [65, q] context
output instead of the big [k, q] probability matrix. Scores for two heads with
the same bias row (h and h+4) are computed concurrently in the PE array via
row tile_position packing (contraction dim is only D=64) and share one big
[128, 2048] exp activation.
"""

from contextlib import ExitStack

import numpy as np

import concourse.bacc as bacc
import concourse.mybir as mybir
import concourse.tile as tile
from concourse.masks import make_identity

F32 = mybir.dt.float32
R32 = mybir.dt.float32r
I32 = mybir.dt.int32
EXP = mybir.ActivationFunctionType.Exp

B, T, HID, H, D = 4, 2048, 1024, 16, 64
TL = T // 2           # query rows owned by one core
N_CORES = 8
NEG_INF = -1.0e9
SCALE = float(D) ** -0.5

IO = HID // 128       # 8 contraction blocks
JBLK = HID // 128     # 8 output-feature blocks
KT = T // 128         # 16 key tiles
TT = TL // 128        # 8 local row tiles
TB = TL // 512        # 2 local row blocks

REPLICA_GROUPS = [[0, 1], [2, 3], [4, 5], [6, 7]]


def _slot(h):
    return h % 4 + 4 * (h // 8)


def _half(h):
    return (h // 4) % 2


def _emit(tc, q_d, k_d, v_d, pm_d, wq_d, wk_d, wv_d, wo_d, out_d):
    nc = tc.nc
    with ExitStack() as ctx:
        const = ctx.enter_context(tc.tile_pool(name="const", bufs=1))
        ident = const.tile([128, 128], F32)
        make_identity(nc, ident)

        # pad mask -> additive bias, laid out [128(k%128), B, KT]
        pm_sb = const.tile([128, B, KT], I32)
        nc.sync.dma_start(pm_sb[:], pm_d.ap().rearrange("b (kt p) -> p b kt", p=128))
        pmf = const.tile([128, B, KT], F32)
        nc.vector.tensor_copy(pmf[:], pm_sb[:])
        biasT = const.tile([128, B, KT], F32)
        nc.vector.tensor_scalar_mul(biasT[:], pmf[:], NEG_INF)
        ones3 = const.tile([128, KT, 1], F32)
        nc.vector.memset(ones3[:], 1.0)

        qpT_pool = ctx.enter_context(tc.tile_pool(name="qpT", bufs=1))
        qpT = qpT_pool.tile([128, 8, TL], R32)   # [u*64+d, slot, q]
        ctxN_pool = ctx.enter_context(tc.tile_pool(name="ctxN", bufs=1))
        ctxN = ctxN_pool.tile([128, JBLK, TL], R32)  # [(h%2)*64+d, h//2, q]

        dram = ctx.enter_context(tc.tile_pool(name="dram", bufs=1, space="DRAM"))
        gath = dram.tile([4 * TL, HID], F32)      # [kpT(g0);vp(g0);kpT(g1);vp(g1)]

        # ---------------- phase A+B: projections ----------------
        with tc.tile_pool(name="w_pool", bufs=2) as wp, \
             tc.tile_pool(name="x_in", bufs=3) as xip, \
             tc.tile_pool(name="xT", bufs=2) as xtp, \
             tc.tile_pool(name="stage", bufs=4) as stp, \
             tc.tile_pool(name="qstage", bufs=3) as qsp, \
             tc.tile_pool(name="ps_t", bufs=2, space="PSUM") as pst, \
             tc.tile_pool(name="ps_p", bufs=3, space="PSUM") as psp:

            def load_w(w_d):
                # chunked per io-block so the first accumulation matmuls can
                # start before the whole 4MB weight arrives
                w = wp.tile([128, IO, HID], R32, tag="w")
                src = w_d.ap().rearrange("(io p) j -> p io j", p=128).bitcast(R32)
                for io in range(IO):
                    nc.sync.dma_start(w[:, io:io + 1, :], src[:, io:io + 1, :])
                return w

            def transpose_slab(x_d, tb):
                # x rows [tb*512, tb*512+512) transposed into [128(i), IO, 512(t)]
                slab = xtp.tile([128, IO, 512], R32, tag="slab")
                for tt4 in range(4):
                    tt = tb * 4 + tt4
                    x_sb = xip.tile([128, HID], F32, tag="x_in")
                    for ih2 in range(2):
                        nc.sync.dma_start(
                            x_sb[:, ih2 * 512:(ih2 + 1) * 512],
                            x_d.ap()[tt * 128:(tt + 1) * 128,
                                     ih2 * 512:(ih2 + 1) * 512])
                    for ih in range(2):
                        ps = pst.tile([128, 512], F32, tag="ps_t")
                        for i4 in range(4):
                            io = ih * 4 + i4
                            nc.tensor.matmul(
                                ps[:, i4 * 128:(i4 + 1) * 128],
                                x_sb[:, io * 128:(io + 1) * 128], ident[:],
                                is_transpose=True,
                                start=(i4 == 0), stop=(i4 == 3))
                        nc.vector.tensor_copy(
                            slab[:, ih * 4:(ih + 1) * 4, tt4 * 128:(tt4 + 1) * 128],
                            ps[:].rearrange("p (a b) -> p a b", a=4))
                return slab

            # k path over the FULL batch (no exchange): kpT halves in gath
            wk = load_w(wk_d)
            for tb in range(2 * TB):
                g, tl = tb // 2, tb % 2
                slab = transpose_slab(k_d, tb)
                for jb in range(JBLK):
                    ps = psp.tile([128, 512], F32, tag="ps_p")
                    for io in range(IO):
                        nc.tensor.matmul(
                            ps[:], wk[:, io, jb * 128:(jb + 1) * 128], slab[:, io, :],
                            start=(io == 0), stop=(io == IO - 1))
                    st = stp.tile([128, 512], F32, tag="stage")
                    nc.vector.tensor_copy(st[:], ps[:])
                    nc.sync.dma_start(
                        gath[g * 2 * TL + jb * 128:g * 2 * TL + (jb + 1) * 128,
                             tl * 512:(tl + 1) * 512], st[:])

            # v path over the FULL batch: vp halves in gath
            wv = load_w(wv_d)
            for tb in range(2 * TB):
                g = tb // 2
                slab = transpose_slab(v_d, tb)
                for tt4 in range(4):
                    ttl = (tb % 2) * 4 + tt4
                    for jh in range(2):
                        ps = psp.tile([128, 512], F32, tag="ps_p")
                        for io in range(IO):
                            nc.tensor.matmul(
                                ps[:], slab[:, io, tt4 * 128:(tt4 + 1) * 128],
                                wv[:, io, jh * 512:(jh + 1) * 512],
                                start=(io == 0), stop=(io == IO - 1))
                        st = stp.tile([128, 512], F32, tag="stage")
                        nc.vector.tensor_copy(st[:], ps[:])
                        nc.sync.dma_start(
                            gath[g * 2 * TL + TL + ttl * 128:
                                 g * 2 * TL + TL + (ttl + 1) * 128,
                                 jh * 512:(jh + 1) * 512], st[:])

            # q path -> resident qpT (overlaps the collective)
            wq = load_w(wq_d)
            for tb in range(TB):
                slab = transpose_slab(q_d, tb)
                for jb in range(JBLK):
                    ps = psp.tile([128, 512], F32, tag="ps_p")
                    for io in range(IO):
                        nc.tensor.matmul(
                            ps[:], wq[:, io, jb * 128:(jb + 1) * 128], slab[:, io, :],
                            start=(io == 0), stop=(io == IO - 1))
                    qs = qsp.tile([128, 512], R32, tag="qstage")
                    nc.vector.tensor_copy(qs[:], ps[:])
                    for r in range(2):
                        h = 2 * jb + r
                        s, u = _slot(h), _half(h)
                        nc.sync.dma_start(
                            qpT[u * 64:(u + 1) * 64, s, tb * 512:(tb + 1) * 512],
                            qs[r * 64:(r + 1) * 64, :])

        # ---------------- phase C: attention ----------------
        gath_r = gath[:].rearrange("(g rr) c -> g rr c", g=2)
        gath_v = gath[:].rearrange("(g half t8 p) c -> g half t8 p c",
                                   g=2, half=2, t8=8)
        # outlives phase C so phase D can consume it; the load overlaps attention
        wop = ctx.enter_context(tc.tile_pool(name="wo_pool", bufs=1))
        wo = wop.tile([128, JBLK, HID], R32)
        nc.sync.dma_start(
            wo[:], wo_d.ap().rearrange("(jo p) n -> p jo n", p=128).bitcast(R32))
        with tc.tile_pool(name="kpTs", bufs=3) as kps_p, \
             tc.tile_pool(name="vpm", bufs=6) as vpm_p, \
             tc.tile_pool(name="pt", bufs=4) as pt_p, \
             tc.tile_pool(name="norm", bufs=4) as nm_p, \
             tc.tile_pool(name="ctmp", bufs=2) as ct_p, \
             tc.tile_pool(name="ps_st", bufs=2, space="PSUM") as ps_st, \
             tc.tile_pool(name="ps_ctx", bufs=2, space="PSUM") as ps_ctx:
            for sp in range(8):
                hA = sp % 4 + 8 * (sp // 4)
                hB = hA + 4
                bb = sp % 4
                kpTs = kps_p.tile([128, T], R32, tag="kpTs")
                for u, h in ((0, hA), (1, hB)):
                    # split per gathered half so MM1 on the first 8 key tiles
                    # starts before the second half of the exchange lands
                    for g in range(2):
                        src = gath_r[g:g + 1, 64 * h:64 * h + 64, :].bitcast(R32)
                        nc.sync.dma_start(
                            kpTs[u * 64:(u + 1) * 64,
                                 g * TL:(g + 1) * TL],
                            src.rearrange("g1 p t -> p (g1 t)"))
                vpms = []
                for h in (hA, hB):
                    vpm = vpm_p.tile([128, KT, 65], R32, tag="vpm")
                    for g in range(2):
                        src = gath_v[g:g + 1, 1:2, :, :, 64 * h:64 * h + 64].bitcast(R32)
                        nc.sync.dma_start(
                            vpm[:, g * 8:(g + 1) * 8, 0:64],
                            src.rearrange("g1 one t8 p j -> p (g1 one t8) j"))
                    nc.vector.tensor_copy(vpm[:, :, 64:65], ones3[:])
                    vpms.append(vpm)
                vpmA, vpmB = vpms
                ctxA = ps_ctx.tile([65, TL], F32, tag="ctx")
                ctxB = ps_ctx.tile([65, TL], F32, tag="ctx")
                for kt in range(KT):
                    for qb in range(2):
                        # scores for both heads of the pair side by side:
                        # same bias row, one exp activation, 2-bank tile that
                        # double-buffers within the 8-bank PSUM budget
                        st = ps_st.tile([128, 1024], F32, tag="st")
                        for u in range(2):
                            nc.tensor.matmul(
                                st[:, u * 512:(u + 1) * 512],
                                kpTs[u * 64:(u + 1) * 64, kt * 128:(kt + 1) * 128],
                                qpT[u * 64:(u + 1) * 64, sp, qb * 512:(qb + 1) * 512],
                                start=True, stop=True,
                                tile_position=(u * 64, 0))
                        pt = pt_p.tile([128, 1024], R32, tag="pt")
                        nc.scalar.activation(pt[:], st[:], EXP,
                                             bias=biasT[:, bb, kt:kt + 1], scale=SCALE)
                        for u, (vpm, ctxZ) in enumerate(((vpmA, ctxA), (vpmB, ctxB))):
                            nc.tensor.matmul(
                                ctxZ[:, qb * 512:(qb + 1) * 512],
                                vpm[:, kt, :],
                                pt[:, u * 512:(u + 1) * 512],
                                start=(kt == 0), stop=(kt == KT - 1))
                # normalize: ctx[d, q] / denom[q]  (denom is matmul row 64)
                for h, ctxZ in ((hA, ctxA), (hB, ctxB)):
                    # NB: partition_broadcast reads physical partition 0 of its
                    # input tile (AP base is ignored), so rden must live at
                    # partition 0; DVE reciprocal handles the 64->0 shift.
                    rden = nm_p.tile([1, TL], F32, tag="rden")
                    nc.vector.reciprocal(rden[:], ctxZ[64:65, :])
                    rdenB = nm_p.tile([64, TL], F32, tag="rdenB")
                    nc.gpsimd.partition_broadcast(rdenB[:], rden[:])
                    jb, r = h // 2, h % 2
                    if r == 0:
                        nc.vector.tensor_mul(
                            ctxN[0:64, jb, :], ctxZ[0:64, :], rdenB[:])
                    else:
                        tmp = ct_p.tile([64, TL], R32, tag="ctmp")
                        nc.vector.tensor_mul(tmp[:], ctxZ[0:64, :], rdenB[:])
                        nc.sync.dma_start(ctxN[64:128, jb, :], tmp[:])

        # ---------------- phase D: output projection ----------------
        with tc.tile_pool(name="ostage", bufs=4) as osp, \
             tc.tile_pool(name="ps_o", bufs=4, space="PSUM") as pso:
            for tt in range(TT):
                for nb in range(2):
                    ps = pso.tile([128, 512], F32, tag="ps_o")
                    for jb in range(JBLK):
                        nc.tensor.matmul(
                            ps[:],
                            ctxN[:, jb, tt * 128:(tt + 1) * 128],
                            wo[:, jb, nb * 512:(nb + 1) * 512],
                            start=(jb == 0), stop=(jb == JBLK - 1))
                    ost = osp.tile([128, 512], F32, tag="ost")
                    nc.vector.tensor_copy(ost[:], ps[:])
                    nc.sync.dma_start(
                        out_d.ap()[tt * 128:(tt + 1) * 128,
                                   nb * 512:(nb + 1) * 512], ost[:])


def build():
    nc = bacc.Bacc("TRN2", target_bir_lowering=False, debug=False,
                   num_devices=N_CORES)
    q_d = nc.dram_tensor("q", [TL, HID], F32, kind="ExternalInput")
    k_d = nc.dram_tensor("k", [T, HID], F32, kind="ExternalInput")
    v_d = nc.dram_tensor("v", [T, HID], F32, kind="ExternalInput")
    pm_d = nc.dram_tensor("pad_mask", [B, T], I32, kind="ExternalInput")
    wq_d = nc.dram_tensor("Wq", [HID, HID], F32, kind="ExternalInput")
    wk_d = nc.dram_tensor("Wk", [HID, HID], F32, kind="ExternalInput")
    wv_d = nc.dram_tensor("Wv", [HID, HID], F32, kind="ExternalInput")
    wo_d = nc.dram_tensor("Wo", [HID, HID], F32, kind="ExternalInput")
    out_d = nc.dram_tensor("out", [TL, HID], F32, kind="ExternalOutput")

    with tile.TileContext(nc) as tc:
        _emit(tc, q_d, k_d, v_d, pm_d, wq_d, wk_d, wv_d, wo_d, out_d)
    nc.compile()
    return nc


_NC = None


def _get_nc():
    global _NC
    if _NC is None:
        _NC = build()
    return _NC


def kernel(**inputs):
    from concourse.bass_utils import run_bass_kernel_spmd

    q = np.ascontiguousarray(np.asarray(inputs["q"], dtype=np.float32))
    k = np.ascontiguousarray(np.asarray(inputs["k"], dtype=np.float32))
    v = np.ascontiguousarray(np.asarray(inputs["v"], dtype=np.float32))
    pm = np.ascontiguousarray(np.asarray(inputs["pad_mask"], dtype=np.int32))
    ws = {n: np.ascontiguousarray(np.asarray(inputs[n], dtype=np.float32))
          for n in ("Wq", "Wk", "Wv", "Wo")}

    in_maps = []
    for c in range(N_CORES):
        b, g = c // 2, c % 2
        sl = slice(g * TL, (g + 1) * TL)
        in_maps.append({
            "q": np.ascontiguousarray(q[b, sl]),
            "k": np.ascontiguousarray(k[b]),
            "v": np.ascontiguousarray(v[b]),
            "pad_mask": pm,
            **ws,
        })
    res = run_bass_kernel_spmd(_get_nc(), in_maps, list(range(N_CORES))).results
    out = np.empty((B, T, HID), np.float32)
    for c in range(N_CORES):
        b, g = c // 2, c % 2
        out[b, g * TL:(g + 1) * TL] = res[c]["out"]
    return out



# revision 9
# speedup vs baseline: 1.0693x; 1.0693x over previous
"""Trainium2 Bass kernel for multi-head attention (B=4, T=2048, HID=1024, H=16, D=64).

Sharding (8 NeuronCores): core c owns batch b = c//2 and head-group hg = c%2
(8 of the 16 heads, i.e. columns [512*hg, 512*hg+512) of Wq/Wk/Wv — the host
slices the weights per core, so the program is uniform). Each core projects
q/k/v for its own 8 heads over the full 2048 rows: projection work is fully
sharded with no redundancy. Attention runs over all 2048 keys for its 8 heads.
Before the output projection, the two cores of a pair exchange normalized
attention-context halves with a per-head-pair AllToAll (256KB each), after
which each core applies the full Wo to all 16 heads for its own query half
g = hg and writes output rows [1024*g, 1024*g+1024).

The exchange is a per-head-pair-slot ReduceScatter(add) over the pair: each
core writes its normalized ctx into a [2 q-half, 2 head-group-slot, 128, 1024]
DRAM buffer with the other head-group slot zeroed (slot selection happens via
a per-core 0/1 mask input, keeping the program SPMD-uniform); the add
interleaves the two head groups and the scatter delivers exactly this core's
query half. Adding an exact bf16 zero is lossless.

Numerics: the host casts q/k/v and the weights to bf16; all matmuls run
bf16 x bf16 -> fp32 PSUM. Inputs are transposed on the fly with the DMA
transpose XBAR (dma_start(transpose=True)) straight from DRAM into SBUF
slabs, so the PE does no transposes at all. The -1e9 pad bias (which, per the
reference's head-major tiling quirk, depends only on h%4) is folded into the
Scalar-engine exp activation; masked keys produce exactly 0. The softmax
denominator comes from a ones-column appended per head to the V operand of
the probability @ V matmul; normalization happens on the [65, q] context.
"""

from contextlib import ExitStack

import numpy as np

import concourse.bacc as bacc
import concourse.mybir as mybir
import concourse.tile as tile

F32 = mybir.dt.float32
BF = mybir.dt.bfloat16
I32 = mybir.dt.int32
EXP = mybir.ActivationFunctionType.Exp

B, T, HID, H, D = 4, 2048, 1024, 16, 64
HH = 8               # heads per core
COLS = HH * D        # 512 hid columns per core
TL = T // 2          # output query rows per core
NSP = 4              # head-pair slots: sp s = local heads (s, s+4)
KT = T // 128        # 16 key tiles
IO = HID // 128      # 8 contraction blocks
NCH = 4              # row chunks of 512 for xbar+projection
N_CORES = 8
NEG_INF = -1.0e9
SCALE = float(D) ** -0.5

REPLICA_GROUPS = [[0, 1], [2, 3], [4, 5], [6, 7]]


def _emit(tc, q_d, k_d, v_d, pm_d, wq_d, wk_d, wv_d, wo_d, msel_d, out_d):
    nc = tc.nc
    with ExitStack() as ctx:
        const = ctx.enter_context(tc.tile_pool(name="const", bufs=1))
        # pad mask -> additive bias, laid out [128(k%128), maskrow, KT]
        pm_sb = const.tile([128, B, KT], I32)
        nc.sync.dma_start(pm_sb[:], pm_d.ap().rearrange("b (kt p) -> p b kt", p=128))
        pmf = const.tile([128, B, KT], F32)
        nc.vector.tensor_copy(pmf[:], pm_sb[:])
        biasT = const.tile([128, B, KT], F32)
        nc.vector.tensor_scalar_mul(biasT[:], pmf[:], NEG_INF)

        # persistent projection outputs
        kqv_pool = ctx.enter_context(tc.tile_pool(name="kqv", bufs=1))
        kpT = kqv_pool.tile([128, NSP, T], BF, tag="kpT")  # [cb-half d, cb, key]
        qpT = kqv_pool.tile([128, NSP, T], BF, tag="qpT")
        vp = kqv_pool.tile([128, KT, HH * 65], BF, tag="vp")  # 64 d + ones per head
        wo_sb = kqv_pool.tile([128, 8, HID], BF, tag="wo")
        ctxG = kqv_pool.tile([128, 2, NSP, TL], BF, tag="ctxG")

        ones_view = vp[:].rearrange("p kt (h e) -> p kt h e", e=65)[:, :, :, 64:65]
        nc.vector.memset(ones_view, 1.0)

        nc.sync.dma_start(
            wo_sb[:], wo_d.ap().rearrange("(m p) n -> p m n", p=128))

        # per-core head-group slot selector: msel[:, slot] is 1.0 iff this
        # core's head-group == slot (host-provided data, uniform program)
        msel = const.tile([128, 2], F32)
        nc.sync.dma_start(msel[:], msel_d.ap().rearrange("s p -> p s"))

        dram = ctx.enter_context(tc.tile_pool(name="dram", bufs=1, space="DRAM"))
        cc_in = [dram.tile([2, 2, 128, TL], BF, tag=f"cci{s}", name=f"cci{s}")
                 for s in range(NSP)]
        cc_out = [dram.tile([2, 128, TL], BF, tag=f"cco{s}", name=f"cco{s}")
                  for s in range(NSP)]

        with tc.tile_pool(name="w_pool", bufs=1) as wp, \
             tc.tile_pool(name="slab", bufs=3) as slp, \
             tc.tile_pool(name="pt", bufs=6) as ptp, \
             tc.tile_pool(name="ctxN", bufs=2) as cnp, \
             tc.tile_pool(name="mkst", bufs=2) as mkp, \
             tc.tile_pool(name="norm", bufs=2) as nmp, \
             tc.tile_pool(name="ps_pj", bufs=2, space="PSUM") as pjp, \
             tc.tile_pool(name="ps_st", bufs=2, space="PSUM") as stp, \
             tc.tile_pool(name="ps_ctx", bufs=1, space="PSUM") as cxp:

            ws = {}
            for nm, w_d in (("k", wk_d), ("q", wq_d), ("v", wv_d)):
                w = wp.tile([128, IO, COLS], BF, tag=f"w{nm}")
                nc.sync.dma_start(
                    w[:], w_d.ap().rearrange("(i p) c -> p i c", p=128))
                ws[nm] = w

            xr = {"k": k_d.ap().rearrange("m (i p) -> m i p", p=128),
                  "q": q_d.ap().rearrange("m (i p) -> m i p", p=128),
                  "v": v_d.ap().rearrange("m (i p) -> m i p", p=128)}

            slabs = {}

            def xbar(nm, c):
                slab = slp.tile([128, IO, 512], BF, tag="slab")
                for i in range(IO):
                    nc.sync.dma_start(
                        slab[:, i, :], xr[nm][c * 512:(c + 1) * 512, i],
                        transpose=True)
                slabs[(nm, c)] = slab

            def proj_kq(nm, dst, c):
                slab = slabs[(nm, c)]
                for cb in range(4):
                    pj = pjp.tile([128, 512], F32, tag="pj")
                    for io in range(IO):
                        nc.tensor.matmul(
                            pj[:], ws[nm][:, io, cb * 128:(cb + 1) * 128],
                            slab[:, io, :], start=(io == 0), stop=(io == IO - 1))
                    nc.vector.tensor_copy(dst[:, cb, c * 512:(c + 1) * 512], pj[:])

            def proj_v(c):
                slab = slabs[("v", c)]
                for kb in range(4):
                    pj = pjp.tile([128, 512], F32, tag="pj")
                    for io in range(IO):
                        nc.tensor.matmul(
                            pj[:], slab[:, io, kb * 128:(kb + 1) * 128],
                            ws["v"][:, io, :], start=(io == 0), stop=(io == IO - 1))
                    kt = c * 4 + kb
                    dst = vp[:, kt, :].rearrange("p (h e) -> p h e", e=65)[:, :, 0:64]
                    nc.vector.tensor_copy(
                        dst, pj[:].rearrange("p (h e) -> p h e", e=64))

            # stage the input transposes; DMA queue order: k, q0, v, q rest
            for c in range(NCH):
                xbar("k", c)
            xbar("q", 0)
            for c in range(NCH):
                xbar("v", c)
            for c in range(1, NCH):
                xbar("q", c)

            # PE order: k proj, q chunk 0, v proj -> attention can start
            for c in range(NCH):
                proj_kq("k", kpT, c)
            proj_kq("q", qpT, 0)
            for c in range(NCH):
                proj_v(c)

            # ---------------- attention ----------------
            for s in range(NSP):
                lhA, lhB = s, s + 4
                cbA, cbB = lhA // 2, lhB // 2
                uA, uB = (lhA % 2) * 64, (lhB % 2) * 64
                ctxN = cnp.tile([128, T], BF, tag="ctxN")
                for qp in range(4):
                    if s == 0 and qp < NCH - 1:
                        proj_kq("q", qpT, qp + 1)
                    ctxA = cxp.tile([65, 512], F32, tag="cA")
                    ctxB = cxp.tile([65, 512], F32, tag="cB")
                    qsl = slice(qp * 512, (qp + 1) * 512)
                    for kt in range(KT):
                        st = stp.tile([128, 1024], F32, tag="st")
                        nc.tensor.matmul(
                            st[:, 0:512],
                            kpT[uA:uA + 64, cbA, kt * 128:(kt + 1) * 128],
                            qpT[uA:uA + 64, cbA, qsl], start=True, stop=True)
                        nc.tensor.matmul(
                            st[:, 512:1024],
                            kpT[uB:uB + 64, cbB, kt * 128:(kt + 1) * 128],
                            qpT[uB:uB + 64, cbB, qsl], start=True, stop=True)
                        pt = ptp.tile([128, 1024], BF, tag="pt")
                        nc.scalar.activation(pt[:], st[:], EXP,
                                             bias=biasT[:, s, kt:kt + 1], scale=SCALE)
                        nc.tensor.matmul(
                            ctxA[:], vp[:, kt, 65 * lhA:65 * lhA + 65],
                            pt[:, 0:512], start=(kt == 0), stop=(kt == KT - 1))
                        nc.tensor.matmul(
                            ctxB[:], vp[:, kt, 65 * lhB:65 * lhB + 65],
                            pt[:, 512:1024], start=(kt == 0), stop=(kt == KT - 1))
                    # normalize this q block: ctx[d, q] / denom[q] (denom = row 64)
                    for ctxZ, base in ((ctxA, 0), (ctxB, 64)):
                        rden = nmp.tile([1, 512], F32, tag="rden")
                        nc.vector.reciprocal(rden[:], ctxZ[64:65, :])
                        rdenB = nmp.tile([64, 512], F32, tag="rdenB")
                        nc.gpsimd.partition_broadcast(rdenB[:], rden[:])
                        if base == 0:
                            nc.vector.tensor_mul(
                                ctxN[0:64, qsl], ctxZ[0:64, :], rdenB[:])
                        else:
                            tmpB = nmp.tile([64, 512], BF, tag="tmpB")
                            nc.vector.tensor_mul(tmpB[:], ctxZ[0:64, :], rdenB[:])
                            nc.sync.dma_start(ctxN[64:128, qsl], tmpB[:])
                    if qp in (1, 3):
                        # stage q-half j = qp//2 into both head-group slots,
                        # masked so only our own slot is nonzero
                        j = qp // 2
                        for slot in range(2):
                            mst = mkp.tile([128, TL], BF, tag="mst")
                            nc.vector.tensor_scalar_mul(
                                mst[:], ctxN[:, j * TL:(j + 1) * TL],
                                msel[:, slot:slot + 1])
                            nc.sync.dma_start(cc_in[s][j, slot], mst[:])
                nc.gpsimd.collective_compute(
                    "ReduceScatter", mybir.AluOpType.add,
                    replica_groups=REPLICA_GROUPS,
                    ins=[cc_in[s].opt()], outs=[cc_out[s].opt()])
                for slot in range(2):
                    nc.sync.dma_start(ctxG[:, slot, s, :], cc_out[s][slot])

        # ---------------- output projection ----------------
        # m-block order: s<3 first so only the last AllToAll gates the tail
        ms = [(j, s) for s in range(NSP) for j in range(2)]
        ms.sort(key=lambda js: (js[1] == NSP - 1, js))
        with tc.tile_pool(name="ostage", bufs=3) as osp, \
             tc.tile_pool(name="ps_o", bufs=2, space="PSUM") as pso:
            for tt in range(TL // 128):
                for nb in range(2):
                    po = pso.tile([128, 512], F32, tag="po")
                    for idx, (j, s) in enumerate(ms):
                        m = 4 * j + s
                        nc.tensor.matmul(
                            po[:], ctxG[:, j, s, tt * 128:(tt + 1) * 128],
                            wo_sb[:, m, nb * 512:(nb + 1) * 512],
                            start=(idx == 0), stop=(idx == len(ms) - 1))
                    ost = osp.tile([128, 512], F32, tag="ost")
                    nc.vector.tensor_copy(ost[:], po[:])
                    nc.sync.dma_start(
                        out_d.ap()[tt * 128:(tt + 1) * 128,
                                   nb * 512:(nb + 1) * 512], ost[:])


def build():
    nc = bacc.Bacc("TRN2", target_bir_lowering=False, debug=False,
                   num_devices=N_CORES)
    q_d = nc.dram_tensor("q", [T, HID], BF, kind="ExternalInput")
    k_d = nc.dram_tensor("k", [T, HID], BF, kind="ExternalInput")
    v_d = nc.dram_tensor("v", [T, HID], BF, kind="ExternalInput")
    pm_d = nc.dram_tensor("pad_mask", [B, T], I32, kind="ExternalInput")
    wq_d = nc.dram_tensor("Wq", [HID, COLS], BF, kind="ExternalInput")
    wk_d = nc.dram_tensor("Wk", [HID, COLS], BF, kind="ExternalInput")
    wv_d = nc.dram_tensor("Wv", [HID, COLS], BF, kind="ExternalInput")
    wo_d = nc.dram_tensor("Wo", [HID, HID], BF, kind="ExternalInput")
    msel_d = nc.dram_tensor("msel", [2, 128], F32, kind="ExternalInput")
    out_d = nc.dram_tensor("out", [TL, HID], F32, kind="ExternalOutput")

    with tile.TileContext(nc) as tc:
        _emit(tc, q_d, k_d, v_d, pm_d, wq_d, wk_d, wv_d, wo_d, msel_d, out_d)
    nc.compile()
    return nc


_NC = None


def _get_nc():
    global _NC
    if _NC is None:
        _NC = build()
    return _NC


def kernel(**inputs):
    import ml_dtypes
    from concourse.bass_utils import run_bass_kernel_spmd

    BF_NP = ml_dtypes.bfloat16
    q = np.asarray(inputs["q"], dtype=np.float32)
    k = np.asarray(inputs["k"], dtype=np.float32)
    v = np.asarray(inputs["v"], dtype=np.float32)
    pm = np.ascontiguousarray(np.asarray(inputs["pad_mask"], dtype=np.int32))
    Wq = np.asarray(inputs["Wq"], dtype=np.float32)
    Wk = np.asarray(inputs["Wk"], dtype=np.float32)
    Wv = np.asarray(inputs["Wv"], dtype=np.float32)
    Wo = np.asarray(inputs["Wo"], dtype=np.float32)

    # Wo rows permuted to the gathered-context head order: block m = 4j + s
    # holds heads (8j + s, 8j + s + 4)
    perm = []
    for m in range(8):
        j, s = m // 4, m % 4
        perm += [8 * j + s, 8 * j + s + 4]
    wo_r = np.ascontiguousarray(
        Wo.reshape(H, D, HID)[perm].reshape(HID, HID).astype(BF_NP))

    xq = [np.ascontiguousarray(q[b].astype(BF_NP)) for b in range(B)]
    xk = [np.ascontiguousarray(k[b].astype(BF_NP)) for b in range(B)]
    xv = [np.ascontiguousarray(v[b].astype(BF_NP)) for b in range(B)]
    whalf = {
        nm: [np.ascontiguousarray(W[:, hg * COLS:(hg + 1) * COLS].astype(BF_NP))
             for hg in range(2)]
        for nm, W in (("Wq", Wq), ("Wk", Wk), ("Wv", Wv))
    }

    in_maps = []
    for c in range(N_CORES):
        b, hg = c // 2, c % 2
        msel = np.zeros((2, 128), dtype=np.float32)
        msel[hg, :] = 1.0
        in_maps.append({
            "q": xq[b], "k": xk[b], "v": xv[b], "pad_mask": pm,
            "Wq": whalf["Wq"][hg], "Wk": whalf["Wk"][hg], "Wv": whalf["Wv"][hg],
            "Wo": wo_r, "msel": msel,
        })
    res = run_bass_kernel_spmd(_get_nc(), in_maps, list(range(N_CORES))).results
    out = np.empty((B, T, HID), np.float32)
    for c in range(N_CORES):
        b, hg = c // 2, c % 2
        out[b, hg * TL:(hg + 1) * TL] = res[c]["out"]
    return out


# revision 16
# speedup vs baseline: 1.1805x; 1.1040x over previous
"""Trainium2 Bass kernel for multi-head attention (B=4, T=2048, HID=1024, H=16, D=64).

Sharding (8 NeuronCores): core c owns batch b = c//2 and head-group hg = c%2
(8 of the 16 heads, i.e. columns [512*hg, 512*hg+512) of Wq/Wk/Wv — the host
slices the weights per core, so the program is uniform). Each core projects
q/k/v for its own 8 heads over the full 2048 rows: projection work is fully
sharded with no redundancy. Attention runs over all 2048 keys for its 8 heads.
Before the output projection, the two cores of a pair exchange normalized
attention-context halves with a per-head-pair AllToAll (256KB each), after
which each core applies the full Wo to all 16 heads for its own query half
g = hg and writes output rows [1024*g, 1024*g+1024).

The exchange is a per-head-pair-slot ReduceScatter(add) over the pair: each
core writes its normalized ctx into a [2 q-half, 2 head-group-slot, 128, 1024]
DRAM buffer with the other head-group slot zeroed (slot selection happens via
a per-core 0/1 mask input, keeping the program SPMD-uniform); the add
interleaves the two head groups and the scatter delivers exactly this core's
query half. Adding an exact bf16 zero is lossless.

Numerics: the host casts q/k/v and the weights to bf16; all matmuls run
bf16 x bf16 -> fp32 PSUM. Inputs are transposed on the fly with the DMA
transpose XBAR (dma_start(transpose=True)) straight from DRAM into SBUF
slabs, so the PE does no transposes at all. The -1e9 pad bias (which, per the
reference's head-major tiling quirk, depends only on h%4) is folded into the
Scalar-engine exp activation; masked keys produce exactly 0. The softmax
denominator comes from a ones-column appended per head to the V operand of
the probability @ V matmul; normalization happens on the [65, q] context.
"""

from contextlib import ExitStack

import numpy as np

import concourse.bacc as bacc
import concourse.mybir as mybir
import concourse.tile as tile

F32 = mybir.dt.float32
BF = mybir.dt.bfloat16
I32 = mybir.dt.int32
EXP = mybir.ActivationFunctionType.Exp

B, T, HID, H, D = 4, 2048, 1024, 16, 64
HH = 8               # heads per core
COLS = HH * D        # 512 hid columns per core
TL = T // 2          # output query rows per core
NSP = 4              # head-pair slots: sp s = local heads (s, s+4)
KT = T // 128        # 16 key tiles
IO = HID // 128      # 8 contraction blocks
NCH = 4              # row chunks of 512 for xbar+projection
N_CORES = 8
NEG_INF = -1.0e9
SCALE = float(D) ** -0.5

REPLICA_GROUPS = [[0, 1], [2, 3], [4, 5], [6, 7]]


def _emit(tc, q_d, k_d, v_d, pm_d, wq_d, wk_d, wv_d, wo_d, msel_d, out_d):
    nc = tc.nc
    with ExitStack() as ctx:
        const = ctx.enter_context(tc.tile_pool(name="const", bufs=1))
        # pad mask -> additive bias, laid out [128(k%128), maskrow, KT]
        pm_sb = const.tile([128, B, KT], I32)
        nc.sync.dma_start(pm_sb[:], pm_d.ap().rearrange("b (kt p) -> p b kt", p=128))
        pmf = const.tile([128, B, KT], F32)
        nc.vector.tensor_copy(pmf[:], pm_sb[:])
        biasT = const.tile([128, B, KT], F32)
        nc.vector.tensor_scalar_mul(biasT[:], pmf[:], NEG_INF)

        # persistent projection outputs
        kqv_pool = ctx.enter_context(tc.tile_pool(name="kqv", bufs=1))
        kpT = kqv_pool.tile([128, NSP, T], BF, tag="kpT")  # [cb-half d, cb, key]
        qpT = kqv_pool.tile([128, NSP, T], BF, tag="qpT")
        vp = kqv_pool.tile([128, KT, HH * 65], BF, tag="vp")  # 64 d + ones per head
        wo_sb = kqv_pool.tile([128, 8, HID], BF, tag="wo")
        ctxG = kqv_pool.tile([128, 2, NSP, TL], BF, tag="ctxG")

        ones_view = vp[:].rearrange("p kt (h e) -> p kt h e", e=65)[:, :, :, 64:65]
        nc.vector.memset(ones_view, 1.0)

        # per-core head-group slot selector: msel[:, slot] is 1.0 iff this
        # core's head-group == slot (host-provided data, uniform program)
        msel = const.tile([128, 2], F32)

        dram = ctx.enter_context(tc.tile_pool(name="dram", bufs=1, space="DRAM"))
        cc_in = [dram.tile([2, 2, 128, TL], BF, tag=f"cci{s}", name=f"cci{s}")
                 for s in range(NSP)]
        cc_out = [dram.tile([2, 128, TL], BF, tag=f"cco{s}", name=f"cco{s}")
                  for s in range(NSP)]

        with tc.tile_pool(name="w_pool", bufs=1) as wp, \
             tc.tile_pool(name="slab", bufs=4) as slp, \
             tc.tile_pool(name="pt", bufs=6) as ptp, \
             tc.tile_pool(name="ctxN", bufs=2) as cnp, \
             tc.tile_pool(name="mkst", bufs=2) as mkp, \
             tc.tile_pool(name="norm", bufs=2) as nmp, \
             tc.tile_pool(name="ps_pj", bufs=2, space="PSUM") as pjp, \
             tc.tile_pool(name="ps_st", bufs=2, space="PSUM") as stp, \
             tc.tile_pool(name="ps_ctx", bufs=1, space="PSUM") as cxp:

            ws = {}
            for nm, w_d in (("k", wk_d), ("q", wq_d), ("v", wv_d)):
                w = wp.tile([128, IO, COLS], BF, tag=f"w{nm}", name=f"w{nm}")
                nc.sync.dma_start(
                    w[:], w_d.ap().rearrange("(i p) c -> p i c", p=128))
                ws[nm] = w

            xr = {"k": k_d.ap().rearrange("m (i p) -> m i p", p=128),
                  "q": q_d.ap().rearrange("m (i p) -> m i p", p=128),
                  "v": v_d.ap().rearrange("m (i p) -> m i p", p=128)}

            slabs = {}

            def xbar(nm, c):
                slab = slp.tile([128, IO, 512], BF, tag="slab", name="slab")
                for i in range(IO):
                    nc.sync.dma_start(
                        slab[:, i, :], xr[nm][c * 512:(c + 1) * 512, i],
                        transpose=True)
                slabs[(nm, c)] = slab

            def proj_kq_tile(nm, dst, c, cb):
                slab = slabs[(nm, c)]
                pj = pjp.tile([128, 512], F32, tag="pj", name="pj")
                for io in range(IO):
                    nc.tensor.matmul(
                        pj[:], ws[nm][:, io, cb * 128:(cb + 1) * 128],
                        slab[:, io, :], start=(io == 0), stop=(io == IO - 1))
                nc.vector.tensor_copy(dst[:, cb, c * 512:(c + 1) * 512], pj[:])

            def proj_v_tile(c, kb):
                slab = slabs[("v", c)]
                pj = pjp.tile([128, 512], F32, tag="pj", name="pj")
                for io in range(IO):
                    nc.tensor.matmul(
                        pj[:], slab[:, io, kb * 128:(kb + 1) * 128],
                        ws["v"][:, io, :], start=(io == 0), stop=(io == IO - 1))
                kt = c * 4 + kb
                dst = vp[:, kt, :].rearrange("p (h e) -> p h e", e=65)[:, :, 0:64]
                nc.vector.tensor_copy(
                    dst, pj[:].rearrange("p (h e) -> p h e", e=64))

            # DMA issue order: only what gates the first scores goes first;
            # wo/msel wait until the queue is clear of critical loads
            xbar("k", 0)
            xbar("q", 0)
            xbar("v", 0)
            for c in range(1, NCH):
                xbar("k", c)
                xbar("v", c)
            for c in range(1, NCH):
                xbar("q", c)
            nc.sync.dma_start(msel[:], msel_d.ap().rearrange("s p -> p s"))
            nc.sync.dma_start(
                wo_sb[:], wo_d.ap().rearrange("(m p) n -> p m n", p=128))

            # minimal PE prologue: chunk 0 of k, q, v; the rest of the
            # projections are injected into sp0's kt loops just in time
            for cb in range(4):
                proj_kq_tile("k", kpT, 0, cb)
            for cb in range(4):
                proj_kq_tile("q", qpT, 0, cb)
            for kb in range(4):
                proj_v_tile(0, kb)

            inject = {}
            for i in range(4):
                inject[(0, 0, 0 + i)] = [lambda i=i: proj_kq_tile("k", kpT, 1, i),
                                         lambda i=i: proj_v_tile(1, i)]
                inject[(0, 0, 4 + i)] = [lambda i=i: proj_kq_tile("k", kpT, 2, i),
                                         lambda i=i: proj_v_tile(2, i)]
                inject[(0, 0, 8 + i)] = [lambda i=i: proj_kq_tile("k", kpT, 3, i),
                                         lambda i=i: proj_v_tile(3, i)]
                inject[(0, 0, 12 + i)] = [lambda i=i: proj_kq_tile("q", qpT, 1, i)]
                inject[(0, 1, 0 + i)] = [lambda i=i: proj_kq_tile("q", qpT, 2, i)]
                inject[(0, 1, 4 + i)] = [lambda i=i: proj_kq_tile("q", qpT, 3, i)]

            # ---------------- attention ----------------
            for s in range(NSP):
                lhA, lhB = s, s + 4
                cbA, cbB = lhA // 2, lhB // 2
                uA, uB = (lhA % 2) * 64, (lhB % 2) * 64
                ctxN = cnp.tile([128, T], BF, tag="ctxN")
                for qp in range(4):
                    ctxA = cxp.tile([65, 512], F32, tag="cA")
                    ctxB = cxp.tile([65, 512], F32, tag="cB")
                    qsl = slice(qp * 512, (qp + 1) * 512)
                    for kt in range(KT):
                        st = stp.tile([128, 1024], F32, tag="st")
                        nc.tensor.matmul(
                            st[:, 0:512],
                            kpT[uA:uA + 64, cbA, kt * 128:(kt + 1) * 128],
                            qpT[uA:uA + 64, cbA, qsl], start=True, stop=True)
                        nc.tensor.matmul(
                            st[:, 512:1024],
                            kpT[uB:uB + 64, cbB, kt * 128:(kt + 1) * 128],
                            qpT[uB:uB + 64, cbB, qsl], start=True, stop=True)
                        pt = ptp.tile([128, 1024], BF, tag="pt")
                        nc.scalar.activation(pt[:], st[:], EXP,
                                             bias=biasT[:, s, kt:kt + 1], scale=SCALE)
                        nc.tensor.matmul(
                            ctxA[:], vp[:, kt, 65 * lhA:65 * lhA + 65],
                            pt[:, 0:512], start=(kt == 0), stop=(kt == KT - 1))
                        nc.tensor.matmul(
                            ctxB[:], vp[:, kt, 65 * lhB:65 * lhB + 65],
                            pt[:, 512:1024], start=(kt == 0), stop=(kt == KT - 1))
                        for thunk in inject.get((s, qp, kt), ()):
                            thunk()
                    # normalize this q block: ctx[d, q] / denom[q] (denom = row 64)
                    for ctxZ, base in ((ctxA, 0), (ctxB, 64)):
                        rden = nmp.tile([1, 512], F32, tag="rden")
                        nc.vector.reciprocal(rden[:], ctxZ[64:65, :])
                        rdenB = nmp.tile([64, 512], F32, tag="rdenB")
                        nc.gpsimd.partition_broadcast(rdenB[:], rden[:])
                        if base == 0:
                            nc.vector.tensor_mul(
                                ctxN[0:64, qsl], ctxZ[0:64, :], rdenB[:])
                        else:
                            tmpB = nmp.tile([64, 512], BF, tag="tmpB")
                            nc.vector.tensor_mul(tmpB[:], ctxZ[0:64, :], rdenB[:])
                            nc.sync.dma_start(ctxN[64:128, qsl], tmpB[:])
                    if qp in (1, 3):
                        # stage q-half j = qp//2 into both head-group slots,
                        # masked so only our own slot is nonzero
                        j = qp // 2
                        for slot in range(2):
                            mst = mkp.tile([128, TL], BF, tag="mst")
                            nc.vector.tensor_scalar_mul(
                                mst[:], ctxN[:, j * TL:(j + 1) * TL],
                                msel[:, slot:slot + 1])
                            nc.sync.dma_start(cc_in[s][j, slot], mst[:])
                nc.gpsimd.collective_compute(
                    "ReduceScatter", mybir.AluOpType.add,
                    replica_groups=REPLICA_GROUPS,
                    ins=[cc_in[s].opt()], outs=[cc_out[s].opt()])

            # readbacks are emitted after ALL attention so no engine queue
            # ever head-blocks on an in-flight collective
            for s in range(NSP):
                for slot in range(2):
                    nc.sync.dma_start(ctxG[:, slot, s, :], cc_out[s][slot])

        # ---------------- output projection ----------------
        # every accumulation group STARTS with an s=3 block (the last
        # collective): the scheduler then cannot hoist phase-D matmuls into
        # the attention stream, where they would stall the in-order PE queue
        ms = [(0, 3), (1, 3)] + [(j, s) for s in range(NSP - 1) for j in range(2)]
        with tc.tile_pool(name="ostage", bufs=3) as osp, \
             tc.tile_pool(name="ps_o", bufs=2, space="PSUM") as pso:
            for tt in range(TL // 128):
                for nb in range(2):
                    po = pso.tile([128, 512], F32, tag="po")
                    for idx, (j, s) in enumerate(ms):
                        m = 4 * j + s
                        nc.tensor.matmul(
                            po[:], ctxG[:, j, s, tt * 128:(tt + 1) * 128],
                            wo_sb[:, m, nb * 512:(nb + 1) * 512],
                            start=(idx == 0), stop=(idx == len(ms) - 1))
                    ost = osp.tile([128, 512], F32, tag="ost")
                    nc.vector.tensor_copy(ost[:], po[:])
                    nc.sync.dma_start(
                        out_d.ap()[tt * 128:(tt + 1) * 128,
                                   nb * 512:(nb + 1) * 512], ost[:])


def build():
    nc = bacc.Bacc("TRN2", target_bir_lowering=False, debug=False,
                   num_devices=N_CORES)
    q_d = nc.dram_tensor("q", [T, HID], BF, kind="ExternalInput")
    k_d = nc.dram_tensor("k", [T, HID], BF, kind="ExternalInput")
    v_d = nc.dram_tensor("v", [T, HID], BF, kind="ExternalInput")
    pm_d = nc.dram_tensor("pad_mask", [B, T], I32, kind="ExternalInput")
    wq_d = nc.dram_tensor("Wq", [HID, COLS], BF, kind="ExternalInput")
    wk_d = nc.dram_tensor("Wk", [HID, COLS], BF, kind="ExternalInput")
    wv_d = nc.dram_tensor("Wv", [HID, COLS], BF, kind="ExternalInput")
    wo_d = nc.dram_tensor("Wo", [HID, HID], BF, kind="ExternalInput")
    msel_d = nc.dram_tensor("msel", [2, 128], F32, kind="ExternalInput")
    out_d = nc.dram_tensor("out", [TL, HID], F32, kind="ExternalOutput")

    with tile.TileContext(nc) as tc:
        _emit(tc, q_d, k_d, v_d, pm_d, wq_d, wk_d, wv_d, wo_d, msel_d, out_d)
    nc.compile()
    return nc


_NC = None


def _get_nc():
    global _NC
    if _NC is None:
        _NC = build()
    return _NC


def kernel(**inputs):
    import ml_dtypes
    from concourse.bass_utils import run_bass_kernel_spmd

    BF_NP = ml_dtypes.bfloat16
    q = np.asarray(inputs["q"], dtype=np.float32)
    k = np.asarray(inputs["k"], dtype=np.float32)
    v = np.asarray(inputs["v"], dtype=np.float32)
    pm = np.ascontiguousarray(np.asarray(inputs["pad_mask"], dtype=np.int32))
    Wq = np.asarray(inputs["Wq"], dtype=np.float32)
    Wk = np.asarray(inputs["Wk"], dtype=np.float32)
    Wv = np.asarray(inputs["Wv"], dtype=np.float32)
    Wo = np.asarray(inputs["Wo"], dtype=np.float32)

    # Wo rows permuted to the gathered-context head order: block m = 4j + s
    # holds heads (8j + s, 8j + s + 4)
    perm = []
    for m in range(8):
        j, s = m // 4, m % 4
        perm += [8 * j + s, 8 * j + s + 4]
    wo_r = np.ascontiguousarray(
        Wo.reshape(H, D, HID)[perm].reshape(HID, HID).astype(BF_NP))

    xq = [np.ascontiguousarray(q[b].astype(BF_NP)) for b in range(B)]
    xk = [np.ascontiguousarray(k[b].astype(BF_NP)) for b in range(B)]
    xv = [np.ascontiguousarray(v[b].astype(BF_NP)) for b in range(B)]
    whalf = {
        nm: [np.ascontiguousarray(W[:, hg * COLS:(hg + 1) * COLS].astype(BF_NP))
             for hg in range(2)]
        for nm, W in (("Wq", Wq), ("Wk", Wk), ("Wv", Wv))
    }

    in_maps = []
    for c in range(N_CORES):
        b, hg = c // 2, c % 2
        msel = np.zeros((2, 128), dtype=np.float32)
        msel[hg, :] = 1.0
        in_maps.append({
            "q": xq[b], "k": xk[b], "v": xv[b], "pad_mask": pm,
            "Wq": whalf["Wq"][hg], "Wk": whalf["Wk"][hg], "Wv": whalf["Wv"][hg],
            "Wo": wo_r, "msel": msel,
        })
    res = run_bass_kernel_spmd(_get_nc(), in_maps, list(range(N_CORES))).results
    out = np.empty((B, T, HID), np.float32)
    for c in range(N_CORES):
        b, hg = c // 2, c % 2
        out[b, hg * TL:(hg + 1) * TL] = res[c]["out"]
    return out
